# revision 20
# baseline (speedup 1.0000x reference)
"""Trainium2 Bass kernel for nn_CML_Model_48859547959346.

The model is a tiny transformer/conv pipeline (n_e=22, A=11, HID=8) whose
output is a single [16] vector x, followed by the memory-bound part:

    psi = Wout @ x + bout      (Wout: [2^22, 16], 256 MB fp32)
    out = psi + bos * 2^(22/2) (bos: kron product of 22 per-qubit 2-vectors)

Strategy (matches the sharding hint):
  * The tiny pipeline reduces to one [16] vector; it is computed on the host
    in float64 (it's a few thousand flops - sub-millisecond) and `bout +
    2048*bos` is folded into a single combined bias vector so the device
    streams no extra data.
  * Wout's 2^22 rows and the output are sharded contiguously across the 8
    NeuronCores (tensor parallel along the 2^qnum dim). Each core computes
    its [524288] slice:  out_c = W_c @ x + bias_c.
  * Per core, the matvec runs on the TensorEngine as 16 accumulating
    matmuls per PSUM tile: lhsT_j = diag(x[j]) (a [128,128] diagonal),
    rhs_j = the stride-16 view W_tile[:, :, j] of the natively-laid-out
    [128, 512*16] SBUF tile.  This keeps the W DMA perfectly contiguous
    (the kernel is purely HBM-bandwidth bound) and produces the output in
    partition-major order so the store DMA is contiguous too.
"""

import math

import numpy as np

HID = 8
QNUM = 22
N_OUT = 1 << QNUM  # 4194304
N_CORES = 8
ROWS_PER_CORE = N_OUT // N_CORES  # 524288
P = 128  # SBUF partitions
F = 512  # output rows per partition per tile
J = 16  # inner (contraction) dim of Wout
TILE_ROWS = P * F  # 65536
N_TILES = ROWS_PER_CORE // TILE_ROWS  # 8


# ----------------------------------------------------------------------------
# Host-side replication of the tiny pipeline (float64 for extra headroom).
# ----------------------------------------------------------------------------

def _ln(x, g, b, eps=1e-5):
    m = np.mean(x, axis=-1, keepdims=True)
    v = np.mean((x - m) ** 2, axis=-1, keepdims=True)
    return (x - m) / np.sqrt(v + eps) * g + b


def _softmax(x, axis=-1):
    m = np.max(x, axis=axis, keepdims=True)
    e = np.exp(x - m)
    return e / np.sum(e, axis=axis, keepdims=True)


def _conv1d_s2(x, w):
    # x: [N, C, L], w: [O, I, K=2], stride 2, VALID, no bias
    L = x.shape[2]
    Lo = (L - 2) // 2 + 1
    x0 = x[:, :, 0 : 2 * Lo : 2]
    x1 = x[:, :, 1 : 2 * Lo : 2]
    return np.einsum("ncl,oc->nol", x0, w[:, :, 0]) + np.einsum(
        "ncl,oc->nol", x1, w[:, :, 1]
    )


def _host_x16_and_bias(inputs, dtype=np.float64):
    f = lambda k: np.asarray(inputs[k], dtype=dtype)
    pos_a = f("pos_a")
    ix_a = np.asarray(inputs["ix_a"])
    pos_ix = np.asarray(inputs["pos_ix"])
    atom_ix = np.asarray(inputs["atom_ix"])
    rpos_w = f("rpos_w")
    emb_w = f("emb_w")
    emb_b = f("emb_b")
    Wq, bq = f("Wq"), f("bq")
    Wk, bk = f("Wk"), f("bk")
    Wv, bv = f("Wv"), f("bv")
    Wo, bo = f("Wo"), f("bo")
    W1, b1 = f("W1"), f("b1")
    W2, b2 = f("W2"), f("b2")
    ln1_g, ln1_b = f("ln1_g"), f("ln1_b")
    ln2_g, ln2_b = f("ln2_g"), f("ln2_b")
    Wi, bi = f("Wi"), f("bi")
    ni_g, ni_b = f("ni_g"), f("ni_b")
    conv_a_w = f("conv_a_w")
    conv_e_w = f("conv_e_w")
    bout = f("bout")

    n_e = pos_ix.shape[0]
    pos_e = rpos_w[pos_ix] + pos_a[atom_ix]  # [n_e, 3]
    ae = pos_e[:, None, :] - pos_a[None, :, :]  # [n_e, A, 3]
    r_ae = np.linalg.norm(ae, axis=2, keepdims=True)  # [n_e, A, 1]
    seq = np.concatenate([ae, r_ae], axis=-1) @ emb_w.T + emb_b  # [n_e, A, HID]
    amp_proto = ix_a.astype(dtype)[None, :, None]
    amp_ae = np.std(r_ae, ddof=1)
    bias_ae = np.mean(r_ae)
    scale = np.sqrt(np.asarray(HID, dtype))
    for l in range(Wq.shape[0]):
        x = amp_proto * seq
        q = x @ Wq[l].T + bq[l]
        k = x @ Wk[l].T + bk[l]
        v = x @ Wv[l].T + bv[l]
        att = _softmax(np.einsum("bqh,bkh->bqk", q, k) / scale, axis=-1)
        a = np.einsum("bqk,bkh->bqh", att, v) @ Wo[l].T + bo[l]
        x = _ln(x + a, ln1_g[l], ln1_b[l])
        h = np.maximum(x @ W1[l].T + b1[l], 0.0) @ W2[l].T + b2[l]
        seq = _ln(x + h, ln2_g[l], ln2_b[l])
    ae_inv = np.linalg.inv(emb_w.T @ emb_w) @ emb_w.T  # [4, HID]
    r = np.einsum("h,bah->ba", ae_inv[-1], seq)[..., None]  # [n_e, A, 1]
    r = amp_ae * (r - np.mean(r)) / np.std(r, ddof=1) + bias_ae
    x = (np.exp(-r) * amp_proto * seq) @ Wi.T + bi  # [n_e, A, 2H]
    x = np.swapaxes(x, -2, -1)  # [n_e, 2H, A]
    y = np.mean(x, axis=-1)  # [n_e, 2H]
    amp_r = np.mean(np.exp(-np.swapaxes(r, -2, -1)), axis=-1)  # [n_e, 1]
    pad = np.zeros((x.shape[0], x.shape[1], 1), x.dtype)
    n_iter_a = (x.shape[-1] + 1) // 2
    for _ in range(n_iter_a):
        x = _conv1d_s2(np.concatenate([x, pad], axis=-1), conv_a_w)
    x = (amp_r * _ln(y + x[..., 0], ni_g, ni_b)).T  # [2H, n_e]
    y = np.mean(x, axis=-1)  # [2H]
    amp_r2 = np.mean(amp_r.T, axis=-1)  # [1]
    x = x[None]  # [1, 2H, n_e]
    pad = np.zeros((1, x.shape[1], 1), x.dtype)
    n_iter_e = (x.shape[-1] + 1) // 2
    for _ in range(n_iter_e):
        x = _conv1d_s2(np.concatenate([x, pad], axis=-1), conv_e_w)
    x16 = amp_r2 * _ln(y + x[0, :, 0], ni_g, ni_b)  # [2H]

    # bos: kron of per-qubit RY(hf_q)|0> amplitudes; hf built at f32 like ref
    hf32 = np.asarray(
        ([math.pi, 0.0] * (n_e // 2)) + [0.0] * (QNUM - n_e), dtype=np.float32
    )
    hf = hf32.astype(dtype)
    c = np.cos(hf / 2.0)
    s = np.sin(hf / 2.0)
    state = np.ones((1,), dtype=dtype)
    for q in range(QNUM):
        state = np.kron(state, np.stack([c[q], s[q]]))
    bias_comb = bout + state * (2.0 ** (QNUM / 2))
    return x16.astype(np.float32), np.ascontiguousarray(bias_comb.astype(np.float32))


# ----------------------------------------------------------------------------
# Device kernel
# ----------------------------------------------------------------------------

_CACHE = {}


BLK = J + 1  # 16 x-blocks + 1 bias block per tile


def _build_bass():
    import concourse.mybir as mybir
    from concourse import bacc
    from concourse.tile import TileContext

    f32 = mybir.dt.float32
    f32r = mybir.dt.float32r
    nc = bacc.Bacc()
    # Host-pretransposed stream: W[t, p, j*F + f] = Wout[row(t,p,f), j] for
    # j < J, and = bias[row(t,p,f)] for j == J.  Fully contiguous DMA, and
    # every matmul rhs slice is a contiguous [128, F] view.  float32r:
    # single-pass fp32 matmul (fp32 proper runs as two half-speed LOW/HIGH
    # passes); measured precision ~1e-6 rel.
    W = nc.dram_tensor("w", [N_TILES, P, BLK * F], f32r, kind="ExternalInput")
    # dx: 16 diag(x[j]) blocks followed by one identity block (for the bias).
    DX = nc.dram_tensor("dx", [P, BLK * P], f32r, kind="ExternalInput")
    OUT = nc.dram_tensor("out", [ROWS_PER_CORE], f32, kind="ExternalOutput")

    O_t = OUT.rearrange("(t p f) -> t p f", t=N_TILES, p=P)

    with TileContext(nc) as tc:
        with (
            tc.tile_pool(name="wpool", bufs=4) as wpool,
            tc.tile_pool(name="opool", bufs=4) as opool,
            tc.tile_pool(name="dxpool", bufs=1) as dxpool,
            tc.tile_pool(name="pspool", bufs=4, space="PSUM") as pspool,
        ):
            dxt = dxpool.tile([P, BLK * P], f32r)
            nc.sync.dma_start(out=dxt[:], in_=DX[:, :])
            for t in range(N_TILES):
                wt = wpool.tile([P, BLK * F], f32r)
                nc.sync.dma_start(out=wt[:], in_=W[t])
                ps = pspool.tile([P, F], f32)
                for j in range(BLK):
                    # psum[m, n] += x[j] * W[row, j]   (j==J: + bias)
                    nc.tensor.matmul(
                        ps[:],
                        dxt[:, j * P : (j + 1) * P],
                        wt[:, j * F : (j + 1) * F],
                        start=(j == 0),
                        stop=(j == BLK - 1),
                    )
                ot = opool.tile([P, F], f32)
                nc.scalar.copy(out=ot[:], in_=ps[:])
                nc.scalar.dma_start(out=O_t[t], in_=ot[:])
    nc.compile()
    return nc


def _get_bass():
    if "nc" not in _CACHE:
        _CACHE["nc"] = _build_bass()
    return _CACHE["nc"]


def _pack_device_inputs(W, bias_comb, x16):
    """Build the per-core device streams.

    wdev[c, t, p, j, f] = W[row, j] for j < J, bias_comb[row] for j == J,
    with row = c*ROWS_PER_CORE + t*TILE_ROWS + p*F + f.
    """
    Wv = W.reshape(N_CORES, N_TILES, P, F, J)
    wdev = np.empty((N_CORES, N_TILES, P, BLK, F), np.float32)
    wdev[:, :, :, :J, :] = np.swapaxes(Wv, 3, 4)
    wdev[:, :, :, J, :] = bias_comb.reshape(N_CORES, N_TILES, P, F)

    diag = np.zeros((P, BLK * P), np.float32)
    idx = np.arange(P)
    for j in range(J):
        diag[idx, j * P + idx] = x16[j]
    diag[idx, J * P + idx] = 1.0  # identity block applies the bias
    return wdev, diag


def _run_device(W, bias_comb, x16, trace=False):
    from concourse.bass_utils import run_bass_kernel_spmd

    wdev, diag = _pack_device_inputs(W, bias_comb, x16)
    in_maps = [
        {"w": wdev[c].reshape(N_TILES, P, BLK * F), "dx": diag}
        for c in range(N_CORES)
    ]
    res = run_bass_kernel_spmd(
        _get_bass(), in_maps, core_ids=list(range(N_CORES)), trace=trace
    )
    out = np.concatenate([res.results[c]["out"] for c in range(N_CORES)])
    return out, res


def kernel(**inputs):
    x16, bias_comb = _host_x16_and_bias(inputs)
    W = np.ascontiguousarray(np.asarray(inputs["Wout"], dtype=np.float32))
    out, _ = _run_device(W, bias_comb, x16, trace=False)
    return out.astype(np.float32, copy=False)
